# revision 4
# baseline (speedup 1.0000x reference)
"""Fused attention-pooling kernel for Trainium2 (Bass/Tile), SPMD over 8 cores.

Reference computation (per batch b):
    uit = tanh(x[b] @ W + bias)          # [T, T]
    s   = uit @ u                        # [T]
    a   = exp(s) * mask[b]               # [T]
    a   = a / (sum(a) + eps)
    out[b] = a @ x[b]                    # [D]

Sharding: data-parallel over the batch dim — 8 batches per NeuronCore.
Layout trick: x is pre-transposed on the host to xT [D, T] per batch so the
big matmul contracts d on the partition dim with no on-device transpose.

Per-core dataflow (T=1024, D=256, 8 batches):
  - main matmul (f32r, full PE rate): zT[k,t] = sum_d W[d,k] * xT[d,t],
    k in 8 chunks of 128 (PSUM [128,1024], accumulated over 2 d-chunks)
  - ACT: uit = tanh(zT + b_k) with fused per-partition bias, PSUM -> SBUF
  - s matmul (f32r): s[1,t] += u_k.T @ uit_k accumulated over k-chunks;
    mask folded in as an additive log-mask (0 / -1e30) via a K=1 matmul
  - ACT: a_raw = exp(s) (f32r, feeds the broadcast matmul directly)
  - DVE: Z = sum(a_raw) + eps, zr = 1/Z collected per batch
  - PE: broadcast a_raw over 128 partitions via ones[1,128].T @ a_raw[1,t]
  - DVE: raw_out[d] = sum_t xT[d,t]*a_bc[d,t] via fused scalar_tensor_tensor
  - end: broadcast all 8 zr values over partitions with one matmul and
    rescale the [128, 16] accumulated outputs
"""

import os
import sys
import functools

import numpy as np

for _p in ("/opt/trn_rl_repo",):
    if _p not in sys.path and os.path.isdir(_p):
        sys.path.insert(0, _p)

B, T, D = 64, 1024, 256
NCORES = 8
BL = B // NCORES          # batches per core
DC = D // 128             # d chunks (2)
KC = T // 128             # k chunks (8)
SEG = 512                 # fp32 matmul moving-dim max (one PSUM bank)
EPS = 1e-7
NEG = -1.0e30             # additive mask: exp(s + NEG) == 0


@functools.cache
def _build():
    import concourse.bacc as bacc
    import concourse.mybir as mybir
    from concourse.tile import TileContext

    f32 = mybir.dt.float32
    f32r = mybir.dt.float32r
    AF = mybir.ActivationFunctionType
    ALU = mybir.AluOpType
    AX = mybir.AxisListType

    nc = bacc.Bacc(
        "TRN2", target_bir_lowering=False, debug=False, num_devices=NCORES
    )
    xT_d = nc.dram_tensor("xT", [BL, DC, 128, T], f32r, kind="ExternalInput").ap()
    w_d = nc.dram_tensor("w", [DC, 128, T], f32r, kind="ExternalInput").ap()
    b_d = nc.dram_tensor("b", [128, KC], f32, kind="ExternalInput").ap()
    u_d = nc.dram_tensor("u", [128, KC], f32r, kind="ExternalInput").ap()
    ones_d = nc.dram_tensor("ones", [1, 128], f32r, kind="ExternalInput").ap()
    lmask_d = nc.dram_tensor("lmask", [1, BL * T], f32r, kind="ExternalInput").ap()
    y_d = nc.dram_tensor("y", [128, BL * DC], f32, kind="ExternalOutput").ap()

    with TileContext(nc) as tc:
        with (
            tc.tile_pool(name="const", bufs=1) as cpool,
            tc.tile_pool(name="xt", bufs=6) as xtp,
            tc.tile_pool(name="uit", bufs=3) as uitp,
            tc.tile_pool(name="veca", bufs=2) as vap,
            tc.tile_pool(name="scr", bufs=2) as scrp,
            tc.tile_pool(name="zp", bufs=2, space="PSUM") as zpp,
            tc.tile_pool(name="sp", bufs=1, space="PSUM") as spp,
            tc.tile_pool(name="abc", bufs=1, space="PSUM") as abcp,
        ):
            w_sb = cpool.tile([128, DC * T], f32r, name="w_sb")
            for c in range(DC):
                nc.sync.dma_start(out=w_sb[:, c * T : (c + 1) * T], in_=w_d[c])
            b_sb = cpool.tile([128, KC], f32, name="b_sb")
            nc.sync.dma_start(out=b_sb[:], in_=b_d[:])
            u_sb = cpool.tile([128, KC], f32r, name="u_sb")
            nc.sync.dma_start(out=u_sb[:], in_=u_d[:])
            ones_sb = cpool.tile([1, 128], f32r, name="ones_sb")
            nc.sync.dma_start(out=ones_sb[:], in_=ones_d[:])
            lmask_sb = cpool.tile([1, BL * T], f32r, name="lmask_sb")
            nc.sync.dma_start(out=lmask_sb[:], in_=lmask_d[:])
            oacc = cpool.tile([128, BL * DC], f32, name="oacc")
            zrs = cpool.tile([1, BL], f32, name="zrs")

            xts = {}
            sps = {}
            araws = {}
            abcs = {}

            def smm(i, kc, uit_t, s_t):
                # s[1, t] += u_kc.T @ uit_kc  (contraction over k on partitions)
                for g in range(T // SEG):
                    nc.tensor.matmul(
                        out=s_t[0:1, g * SEG : (g + 1) * SEG],
                        lhsT=u_sb[:, kc : kc + 1],
                        rhs=uit_t[:, g * SEG : (g + 1) * SEG],
                        start=(kc == 0),
                        stop=False,
                    )

            def lmaskmm(i, s_t):
                # s += log-mask (0 or -1e30), K=1 matmul; closes the group
                for g in range(T // SEG):
                    nc.tensor.matmul(
                        out=s_t[0:1, g * SEG : (g + 1) * SEG],
                        lhsT=ones_sb[0:1, 0:1],
                        rhs=lmask_sb[0:1, i * T + g * SEG : i * T + (g + 1) * SEG],
                        start=False,
                        stop=True,
                    )

            def epilogue_a(i):
                # a_raw = exp(s) in f32r (feeds PE); Z = sum + eps; zr = 1/Z
                s_t = sps.pop(i)
                a_t = vap.tile([1, T], f32r, name=f"a_{i}", tag="a")
                nc.scalar.activation(a_t[:], s_t[:], AF.Exp)
                araws[i] = a_t
                zz = vap.tile([1, 1], f32, name=f"zz_{i}", tag="zz")
                nc.vector.reduce_sum(zz[:], a_t[:].bitcast(f32), axis=AX.X)
                zz2 = vap.tile([1, 1], f32, name=f"zz2_{i}", tag="zz2")
                nc.vector.tensor_scalar_add(zz2[:], zz[:], EPS)
                nc.vector.reciprocal(zrs[0:1, i : i + 1], zz2[:])

            def epilogue_b(i):
                # broadcast a_raw over partitions: ones[1,128].T @ a[1,t]
                a_t = araws.pop(i)
                abc_t = abcp.tile([128, T], f32, name=f"abc_{i}", tag="abc")
                for g in range(T // SEG):
                    nc.tensor.matmul(
                        out=abc_t[:, g * SEG : (g + 1) * SEG],
                        lhsT=ones_sb[:],
                        rhs=a_t[0:1, g * SEG : (g + 1) * SEG],
                        start=True,
                        stop=True,
                    )
                abcs[i] = abc_t

            def epilogue_c(i):
                # raw_out[d] = sum_t xT[d, t] * a_bc[d, t]
                abc_t = abcs.pop(i)
                for c, xt in enumerate(xts.pop(i)):
                    scr_t = scrp.tile([128, T], f32, name=f"scr_{i}_{c}", tag="scr")
                    nc.vector.scalar_tensor_tensor(
                        out=scr_t[:],
                        in0=xt[:].bitcast(f32),
                        scalar=1.0,
                        in1=abc_t[:],
                        op0=ALU.mult,
                        op1=ALU.mult,
                        accum_out=oacc[:, i * DC + c : i * DC + c + 1],
                    )

            for i in range(BL):
                xt_i = []
                for c in range(DC):
                    xt_c = xtp.tile([128, T], f32r, name=f"xt{i}_{c}", tag="xt")
                    nc.sync.dma_start(out=xt_c[:], in_=xT_d[i, c])
                    xt_i.append(xt_c)
                xts[i] = tuple(xt_i)
                s_t = spp.tile([1, T], f32, name=f"s_{i}", tag="s")
                sps[i] = s_t
                uits = {}
                for kc in range(KC):
                    zp_t = zpp.tile([128, T], f32, name=f"zp_{i}_{kc}", tag="zp")
                    for c in range(DC):
                        for g in range(T // SEG):
                            nc.tensor.matmul(
                                out=zp_t[:, g * SEG : (g + 1) * SEG],
                                lhsT=w_sb[
                                    :, c * T + kc * 128 : c * T + (kc + 1) * 128
                                ],
                                rhs=xt_i[c][:, g * SEG : (g + 1) * SEG],
                                start=(c == 0),
                                stop=(c == DC - 1),
                            )
                    uit_t = uitp.tile([128, T], f32r, name=f"uit_{i}_{kc}", tag="uit")
                    nc.scalar.activation(
                        uit_t[:], zp_t[:], AF.Tanh, bias=b_sb[:, kc : kc + 1]
                    )
                    uits[kc] = uit_t
                    if kc >= 2:
                        smm(i, kc - 2, uits[kc - 2], s_t)
                    if i >= 1:
                        if kc == 0:
                            epilogue_a(i - 1)
                        elif kc == 2:
                            epilogue_b(i - 1)
                        elif kc == 3:
                            epilogue_c(i - 1)
                smm(i, KC - 2, uits[KC - 2], s_t)
                smm(i, KC - 1, uits[KC - 1], s_t)
                lmaskmm(i, s_t)

            epilogue_a(BL - 1)
            epilogue_b(BL - 1)
            epilogue_c(BL - 1)

            # one-shot normalization: broadcast the 8 reciprocals over the
            # partition dim (plain fp32 matmul — no f32r rounding constraint),
            # then scale each batch's [128, 2] output columns
            zrb_t = spp.tile([128, BL], f32, name="zrb", tag="s")
            nc.tensor.matmul(
                out=zrb_t[:],
                lhsT=ones_sb[:].bitcast(f32),
                rhs=zrs[0:1, :].bitcast(f32),
                start=True,
                stop=True,
            )
            for i in range(BL):
                nc.vector.tensor_scalar_mul(
                    oacc[:, i * DC : (i + 1) * DC],
                    oacc[:, i * DC : (i + 1) * DC],
                    zrb_t[:, i : i + 1],
                )
            nc.sync.dma_start(out=y_d[:], in_=oacc[:])

    nc.compile()
    return nc


def _prep_inputs(x, W, b, u, mask):
    x = np.asarray(x, dtype=np.float32)
    W = np.asarray(W, dtype=np.float32)
    b = np.asarray(b, dtype=np.float32)
    u = np.asarray(u, dtype=np.float32)
    mask = np.asarray(mask)

    xT = np.ascontiguousarray(
        x.reshape(NCORES, BL, T, D).transpose(0, 1, 3, 2)
    ).reshape(NCORES, BL, DC, 128, T)
    w_in = np.ascontiguousarray(W.reshape(DC, 128, T))
    b_in = np.ascontiguousarray(b.reshape(KC, 128).T)
    u_in = np.ascontiguousarray(u.reshape(T).reshape(KC, 128).T)
    ones_in = np.ones((1, 128), np.float32)
    lmask = np.where(mask, np.float32(0.0), np.float32(NEG)).astype(np.float32)
    lmask_in = np.ascontiguousarray(lmask.reshape(NCORES, 1, BL * T))
    return [
        {
            "xT": xT[c],
            "w": w_in,
            "b": b_in,
            "u": u_in,
            "ones": ones_in,
            "lmask": lmask_in[c],
        }
        for c in range(NCORES)
    ]


def _assemble(results):
    outs = []
    for c in range(NCORES):
        yc = results[c]["y"]  # [128, BL*DC]
        yc = yc.reshape(128, BL, DC).transpose(1, 2, 0).reshape(BL, D)
        outs.append(yc)
    return np.ascontiguousarray(np.concatenate(outs, 0), dtype=np.float32)


def kernel(x, W, b, u, mask):
    from concourse.bass_utils import run_bass_kernel_spmd

    nc = _build()
    in_maps = _prep_inputs(x, W, b, u, mask)
    res = run_bass_kernel_spmd(nc, in_maps, list(range(NCORES)))
    return _assemble(res.results)
